# revision 1
# baseline (speedup 1.0000x reference)
"""GCN message-passing Bass kernel for TRN2 (8 cores).

Math: delta = segment_sum(w_e * x[src_e]) @ W^T   (linearity: transform after aggregate)

Sharding: targets split across 8 cores (12500 each). Per core, targets are
degree-sorted and grouped into 128-target blocks; each block-j target p has
D_j padded edge slots. One indirect DMA gathers x rows for a whole block:
out[p, d, :] = x[idx[p, d], :]  (pad slots point at row 0, weight 0).
DVE multiplies by per-slot weights (broadcast AP) and reduces over slots.
PE transposes agg and applies W^T; indirect DMA scatters final rows.
"""

import math
from contextlib import ExitStack

import numpy as np

import concourse.bass as bass
import concourse.bacc as bacc
import concourse.mybir as mybir
import concourse.tile as tile
from concourse.bass import IndirectOffsetOnAxis
from concourse.bass_utils import run_bass_kernel_spmd

P = 128
N_CORES = 8
F32 = mybir.dt.float32
I32 = mybir.dt.int32


def preprocess(source, target, edge_weights, n_nodes, n_cores=N_CORES):
    """Build per-core gather/weight/target-id arrays and the shared block schedule.

    Returns dict with:
      d_sched: list[int] per-block slot count (same for all cores)
      per_core: list of dicts with idx_all [128,S] i32, w_all [128,S] f32,
                tgt_all [128,nblk] i32
      nt: targets per core, nblk: blocks per core
    """
    source = np.asarray(source).astype(np.int64)
    target = np.asarray(target).astype(np.int64)
    edge_weights = np.asarray(edge_weights).astype(np.float32)
    nt = n_nodes // n_cores
    assert nt * n_cores == n_nodes
    nblk = math.ceil(nt / P)

    cores = []
    for k in range(n_cores):
        lo, hi = k * nt, (k + 1) * nt
        m = (target >= lo) & (target < hi)
        src_k = source[m]
        w_k = edge_weights[m]
        tl_k = target[m] - lo  # local target ids

        deg = np.bincount(tl_k, minlength=nt)
        perm = np.argsort(deg, kind="stable")  # local ids, degree-ascending
        # block j holds targets perm[j*128:(j+1)*128]; slot count = max degree in block
        deg_sorted = deg[perm]
        d_k = [int(deg_sorted[j * P : (j + 1) * P].max()) if j * P < nt else 0
               for j in range(nblk)]
        cores.append(dict(src=src_k, w=w_k, tl=tl_k, deg=deg, perm=perm, d_k=d_k))

    d_sched = [max(c["d_k"][j] for c in cores) for j in range(nblk)]
    S = sum(d_sched)
    offs = np.concatenate([[0], np.cumsum(d_sched)]).astype(np.int64)

    per_core = []
    for k in range(n_cores):
        c = cores[k]
        # CSR by local target id
        order = np.argsort(c["tl"], kind="stable")
        src_s, w_s = c["src"][order], c["w"][order]
        starts = np.concatenate([[0], np.cumsum(c["deg"])]).astype(np.int64)

        # Paired gather: idx addresses row-pairs of x viewed as [n/2, 128];
        # each slot expands to two weight columns (even/odd row of the pair).
        idx_all = np.zeros((P, S), dtype=np.int32)  # pad -> pair 0 (weights 0)
        w_all = np.zeros((P, 2 * S), dtype=np.float32)
        tgt_all = np.full((P, nblk), 1 << 20, dtype=np.int32)  # pad -> OOB skip
        perm = c["perm"]
        for j in range(nblk):
            o = offs[j]
            blk = perm[j * P : (j + 1) * P]
            for p, t in enumerate(blk):
                s0, d = starts[t], c["deg"][t]
                if d:
                    sl = src_s[s0 : s0 + d]
                    idx_all[p, o : o + d] = sl >> 1
                    w_all[p, 2 * o + 2 * np.arange(d) + (sl & 1)] = w_s[s0 : s0 + d]
                tgt_all[p, j] = t
        per_core.append(dict(idx_all=idx_all, w_all=w_all, tgt_all=tgt_all))

    return dict(d_sched=d_sched, S=S, per_core=per_core, nt=nt, nblk=nblk)


def build_nc(d_sched, S, n_nodes, nt, nblk, d_feat=64, bufs=3):
    nc = bacc.Bacc("TRN2", target_bir_lowering=False, debug=False)
    D = d_feat
    x_t = nc.dram_tensor("x", [n_nodes // 2, 2 * D], F32, kind="ExternalInput")
    wt_t = nc.dram_tensor("wT", [D, D], F32, kind="ExternalInput")
    idx_t = nc.dram_tensor("idx", [P, S], I32, kind="ExternalInput")
    wgt_t = nc.dram_tensor("wgt", [P, 2 * S], F32, kind="ExternalInput")
    tgt_t = nc.dram_tensor("tgt", [P, nblk], I32, kind="ExternalInput")
    eye_t = nc.dram_tensor("eye", [P, P], F32, kind="ExternalInput")
    out_t = nc.dram_tensor("out", [nt, D], F32, kind="ExternalOutput")

    with tile.TileContext(nc) as tc, ExitStack() as ctx:
        const = ctx.enter_context(tc.tile_pool(name="const", bufs=1))
        gpool = ctx.enter_context(tc.tile_pool(name="gather", bufs=bufs))
        mpool = ctx.enter_context(tc.tile_pool(name="msg", bufs=bufs))
        apool = ctx.enter_context(tc.tile_pool(name="agg", bufs=bufs))
        tpool = ctx.enter_context(tc.tile_pool(name="aggT", bufs=bufs))
        dpool = ctx.enter_context(tc.tile_pool(name="delta", bufs=bufs))
        psum = ctx.enter_context(tc.tile_pool(name="psum", bufs=4, space="PSUM"))

        ident = const.tile([P, P], F32)
        nc.sync.dma_start(out=ident[:], in_=eye_t.ap())
        wt_sb = const.tile([D, D], F32)
        nc.sync.dma_start(out=wt_sb[:], in_=wt_t.ap())
        idx_sb = const.tile([P, S], I32)
        nc.sync.dma_start(out=idx_sb[:], in_=idx_t.ap())
        wgt_sb = const.tile([P, 2 * S], F32)
        nc.sync.dma_start(out=wgt_sb[:], in_=wgt_t.ap())
        tgt_sb = const.tile([P, nblk], I32)
        nc.sync.dma_start(out=tgt_sb[:], in_=tgt_t.ap())
        bounds_reg = nc.gpsimd.to_reg(nt - 1)

        # Prime engines on the upfront loads so per-block instructions carry
        # at most one sync wait each (SEQ instruction structs encode one).
        prime = const.tile([P, 1], F32)
        nc.vector.tensor_copy(out=prime[:], in_=wgt_sb[:, :1])
        prime_ps = psum.tile([P, P], F32, tag="tp")
        nc.tensor.transpose(out=prime_ps[:], in_=ident[:], identity=ident[:])
        nc.tensor.transpose(out=prime_ps[:D, :D], in_=wt_sb[:], identity=ident[:D, :D])

        off = 0
        for j in range(nblk):
            dj = d_sched[j]
            agg = apool.tile([P, D], F32, tag="agg")
            if dj > 0:
                g = gpool.tile([P, dj * 2 * D], F32, tag="g")
                for dd in range(dj):
                    nc.gpsimd.indirect_dma_start(
                        out=g[:, dd * 2 * D : (dd + 1) * 2 * D],
                        out_offset=None,
                        in_=x_t.ap(),
                        in_offset=IndirectOffsetOnAxis(
                            ap=idx_sb[:, off + dd : off + dd + 1], axis=0
                        ),
                    )
                msg = mpool.tile([P, dj * 2 * D], F32, tag="m")
                nc.vector.tensor_tensor(
                    out=msg[:].rearrange("p (d o) -> p d o", o=D),
                    in0=g[:].rearrange("p (d o) -> p d o", o=D),
                    in1=wgt_sb[:, 2 * off : 2 * (off + dj)].to_broadcast(
                        [P, 2 * dj, D]
                    ),
                    op=mybir.AluOpType.mult,
                )
                nc.vector.tensor_reduce(
                    out=agg[:],
                    in_=msg[:].rearrange("p (d o) -> p o d", o=D),
                    axis=mybir.AxisListType.X,
                    op=mybir.AluOpType.add,
                )
            else:
                nc.vector.memset(agg[:], 0.0)

            agg_ps = psum.tile([D, P], F32, tag="tp")
            nc.tensor.transpose(out=agg_ps[:], in_=agg[:], identity=ident[:])
            agg_tr = tpool.tile([D, P], F32, tag="aT")
            nc.vector.tensor_copy(out=agg_tr[:], in_=agg_ps[:])

            d_ps = psum.tile([P, D], F32, tag="mm")
            nc.tensor.matmul(out=d_ps[:], lhsT=agg_tr[:], rhs=wt_sb[:], start=True, stop=True)
            d_sb = dpool.tile([P, D], F32, tag="d")
            nc.vector.tensor_copy(out=d_sb[:], in_=d_ps[:])

            nc.gpsimd.indirect_dma_start(
                out=out_t.ap(),
                out_offset=IndirectOffsetOnAxis(ap=tgt_sb[:, j : j + 1], axis=0),
                in_=d_sb[:],
                in_offset=None,
                bounds_check=bounds_reg,
                oob_is_err=False,
            )
            off += dj
    nc.compile()
    return nc


def run_gcn(x, W, edge_weights, source, target, num_nodes, trace=False, bufs=3):
    """Full-input host entry: preprocess, build, run on 8 cores, assemble output."""
    n_nodes = int(num_nodes)
    pp = preprocess(source, target, edge_weights, n_nodes)
    nc = build_nc(pp["d_sched"], pp["S"], n_nodes, pp["nt"], pp["nblk"],
                  d_feat=x.shape[1], bufs=bufs)
    x_np = np.ascontiguousarray(np.asarray(x), dtype=np.float32).reshape(
        n_nodes // 2, 2 * x.shape[1]
    )
    wt_np = np.ascontiguousarray(np.asarray(W).T, dtype=np.float32)
    in_maps = []
    for k in range(N_CORES):
        pc = pp["per_core"][k]
        in_maps.append({
            "x": x_np, "wT": wt_np, "eye": np.eye(P, dtype=np.float32),
            "idx": pc["idx_all"], "wgt": pc["w_all"], "tgt": pc["tgt_all"],
        })
    res = run_bass_kernel_spmd(nc, in_maps, core_ids=list(range(N_CORES)), trace=trace)
    out = np.concatenate([res.results[k]["out"] for k in range(N_CORES)], axis=0)
    return out, res


def kernel(**inputs) -> np.ndarray:
    """Harness entry: full unsharded inputs -> full (num_nodes, 64) output."""
    out, _ = run_gcn(
        np.asarray(inputs["x"]),
        np.asarray(inputs["W"]),
        np.asarray(inputs["edge_weights"]),
        np.asarray(inputs["source"]),
        np.asarray(inputs["target"]),
        int(inputs["num_nodes"]),
        trace=False,
    )
    return out



# revision 4
# speedup vs baseline: 11.8754x; 11.8754x over previous
"""GCN message-passing Bass kernel for TRN2 (8 cores), v2.

Math: delta = segment_sum(w_e * x[src_e]) @ W^T  (transform after aggregate).

Sharding: targets split across 8 cores (12500 each), then into 4 shards of
3125 targets per core.  Per shard, targets are degree-sorted (descending)
and grouped into 128-target blocks; block j gets d_j padded slot columns in
a [128 lanes, S] slot grid (lane = target, column = edge slot).

The gather uses the Q7 ucode `dma_gather` instruction: ONE instruction moves
up to 31 grid columns (31*128 = 3968 rows of 256 B) from a per-shard
host-compacted copy of x (unique source rows, < 32768 so indices fit int16).
This amortizes the 994 ns SWDGE fixed cost that dominated the per-slot
indirect-DMA baseline.  dst[i%128, i//128] = src[idx[i]] exactly matches the
slot grid when the host orders the index list column-major.

DVE multiplies by per-slot weights and reduces each block to agg [128, 64];
PE transposes and applies W^T; the Activation engine does PSUM->SBUF copies.
Output rows go out via one `dma_scatter_add` per shard (int16 core-local
target ids, trailing -1 padding; output buffers are pre-zeroed by the
runtime so add == write).

Known device limits (found empirically): a single dma_gather hangs above 31
dst columns, and single_packet=True deadlocks when descriptors exceed the
SWDGE ring (1024), so all gathers/scatters use single_packet=False.
"""

import math
from contextlib import ExitStack

import numpy as np

import concourse.bass as bass
import concourse.bacc as bacc
import concourse.mybir as mybir
import concourse.tile as tile
from concourse.bass_utils import run_bass_kernel_spmd

P = 128
N_CORES = 8
NSH = 4                # target shards per core (int16 gather-index limit)
D = 64
GCAP = 31              # max slot columns per dma_gather (device limit)
F32 = mybir.dt.float32
I16 = mybir.dt.int16


def _wrap16(v):
    """Index list -> [16, n/16] wrap (element i at [i%16, i//16]), tiled to
    128 partitions (8 Q7 core replicas)."""
    v = np.asarray(v, dtype=np.int16)
    n = len(v)
    assert n % 16 == 0
    w = np.zeros((16, n // 16), dtype=np.int16)
    w[np.arange(n) % 16, np.arange(n) // 16] = v
    return np.tile(w, (8, 1))


def preprocess(x, source, target, edge_weights, n_nodes):
    """Build the shared block/group schedule and per-core tensors.

    Returns dict with:
      d_sched [NSH][nblk], groups[s] = list of (block_start, nblocks, slot_off,
      width), S[s], ncomp[s] (shared padded row counts), and per-core dicts of
      xc{s}, idx{s}, wgt{s}, sidx{s} arrays.
    """
    x = np.asarray(x, dtype=np.float32)
    source = np.asarray(source).astype(np.int64)
    target = np.asarray(target).astype(np.int64)
    edge_weights = np.asarray(edge_weights).astype(np.float32)
    nt = n_nodes // N_CORES
    assert nt * N_CORES == n_nodes
    tps = nt // NSH
    assert tps * NSH == nt
    nblk = math.ceil(tps / P)

    # ---- pass 1: shared schedule (max over cores per block) ----
    per_core = []
    d_sched = np.zeros((NSH, nblk), dtype=np.int64)
    for k in range(N_CORES):
        lo = k * nt
        m = (target >= lo) & (target < lo + nt)
        tl = target[m] - lo
        shards = []
        for s in range(NSH):
            ms = (tl >= s * tps) & (tl < (s + 1) * tps)
            tid = tl[ms] - s * tps
            src_s = source[m][ms]
            w_s = edge_weights[m][ms]
            deg = np.bincount(tid, minlength=tps)
            perm = np.argsort(-deg, kind="stable")
            degs = deg[perm]
            for j in range(nblk):
                hi = min((j + 1) * P, tps)
                dj = int(degs[j * P : hi].max()) if j * P < tps else 0
                d_sched[s, j] = max(d_sched[s, j], dj)
            shards.append(dict(tid=tid, src=src_s, w=w_s, deg=deg, perm=perm))
        per_core.append(shards)

    S = d_sched.sum(axis=1)
    # gather groups: consecutive blocks, at most GCAP slot columns each
    groups = []
    for s in range(NSH):
        gs, b0, width, off = [], 0, 0, 0
        for j in range(nblk):
            dj = int(d_sched[s, j])
            assert 0 < dj <= GCAP
            if width + dj > GCAP:
                gs.append((b0, j - b0, off, width))
                off += width
                b0, width = j, dj
            else:
                width += dj
        gs.append((b0, nblk - b0, off, width))
        groups.append(gs)

    col_off = np.concatenate(
        [np.zeros((NSH, 1), dtype=np.int64), np.cumsum(d_sched, axis=1)], axis=1
    )

    # ---- pass 2: per-core compacted x + index/weight grids ----
    ncomp = np.zeros(NSH, dtype=np.int64)
    raw = []
    for k in range(N_CORES):
        rows = []
        for s in range(NSH):
            sh = per_core[k][s]
            uniq = np.unique(sh["src"])
            assert len(uniq) <= 32767, len(uniq)
            ncomp[s] = max(ncomp[s], len(uniq))
            rows.append(uniq)
        raw.append(rows)

    in_maps = []
    for k in range(N_CORES):
        im = {}
        for s in range(NSH):
            sh = per_core[k][s]
            uniq = raw[k][s]
            Ss = int(S[s])

            xc = np.zeros((int(ncomp[s]), D), dtype=np.float32)
            xc[: len(uniq)] = x[uniq]
            im[f"xc{s}"] = xc

            # CSR by shard-local target id
            order = np.argsort(sh["tid"], kind="stable")
            tid_s = sh["tid"][order]
            src_s = np.searchsorted(uniq, sh["src"][order]).astype(np.int64)
            w_s = sh["w"][order]
            starts = np.concatenate([[0], np.cumsum(sh["deg"])]).astype(np.int64)
            r = np.arange(len(tid_s)) - starts[tid_s]  # rank within target

            lane = np.empty(tps, dtype=np.int64)
            blk = np.empty(tps, dtype=np.int64)
            lane[sh["perm"]] = np.arange(tps) % P
            blk[sh["perm"]] = np.arange(tps) // P

            gidx = np.zeros((P, Ss), dtype=np.int16)  # pad -> row 0, weight 0
            gw = np.zeros((P, Ss), dtype=np.float32)
            cols = col_off[s, blk[tid_s]] + r
            gidx[lane[tid_s], cols] = src_s.astype(np.int16)
            gw[lane[tid_s], cols] = w_s

            # gather index list, column-major over the grid, wrapped per group
            glist = gidx.T.ravel()  # position col*128+lane
            im[f"idx{s}"] = np.concatenate(
                [_wrap16(glist[o * P : (o + w) * P]) for (_, _, o, w) in groups[s]],
                axis=1,
            )
            im[f"wgt{s}"] = gw

            # scatter ids: position j*128+p -> core-local target id; pads (-1)
            # only in the trailing lanes of the last block
            sid = np.full(nblk * P, -1, dtype=np.int16)
            ids = s * tps + sh["perm"]
            pos = blk[sh["perm"]] * P + lane[sh["perm"]]
            sid[pos] = ids.astype(np.int16)
            assert (sid[:tps] >= 0).all() and (sid[tps:] == -1).all()
            im[f"sidx{s}"] = _wrap16(sid)
        in_maps.append(im)

    return dict(
        d_sched=d_sched, groups=groups, S=S, ncomp=ncomp, nt=nt, tps=tps,
        nblk=nblk, in_maps=in_maps,
    )


def build_nc(d_sched, groups, S, ncomp, nt, tps, nblk, bufs=3):
    nc = bacc.Bacc("TRN2", target_bir_lowering=False, debug=False)
    xc_t = [nc.dram_tensor(f"xc{s}", [int(ncomp[s]), D], F32, kind="ExternalInput")
            for s in range(NSH)]
    idx_t = [nc.dram_tensor(f"idx{s}", [P, 8 * int(S[s])], I16, kind="ExternalInput")
             for s in range(NSH)]
    wgt_t = [nc.dram_tensor(f"wgt{s}", [P, int(S[s])], F32, kind="ExternalInput")
             for s in range(NSH)]
    sidx_t = [nc.dram_tensor(f"sidx{s}", [P, 8 * nblk], I16, kind="ExternalInput")
              for s in range(NSH)]
    wt_t = nc.dram_tensor("wT", [D, D], F32, kind="ExternalInput")
    eye_t = nc.dram_tensor("eye", [P, P], F32, kind="ExternalInput")
    out_t = nc.dram_tensor("out", [nt, D], F32, kind="ExternalOutput")

    with tile.TileContext(nc) as tc, ExitStack() as ctx:
        const = ctx.enter_context(tc.tile_pool(name="const", bufs=1))
        gpool = ctx.enter_context(tc.tile_pool(name="gather", bufs=bufs))
        mpool = ctx.enter_context(tc.tile_pool(name="msg", bufs=bufs))
        apool = ctx.enter_context(tc.tile_pool(name="agg", bufs=bufs))
        tpool = ctx.enter_context(tc.tile_pool(name="aggT", bufs=bufs))
        dpool = ctx.enter_context(tc.tile_pool(name="delta", bufs=2))
        psum = ctx.enter_context(tc.tile_pool(name="psum", bufs=4, space="PSUM"))

        ident = const.tile([P, P], F32, tag="eye")
        nc.sync.dma_start(out=ident[:], in_=eye_t.ap())
        wt_sb = const.tile([D, D], F32, tag="wt")
        nc.sync.dma_start(out=wt_sb[:], in_=wt_t.ap())
        idx_sb, wgt_sb, sidx_sb = [], [], []
        for s in range(NSH):
            t = const.tile([P, 8 * int(S[s])], I16, tag=f"idx{s}")
            nc.sync.dma_start(out=t[:], in_=idx_t[s].ap())
            idx_sb.append(t)
            t = const.tile([P, int(S[s])], F32, tag=f"wgt{s}")
            nc.sync.dma_start(out=t[:], in_=wgt_t[s].ap())
            wgt_sb.append(t)
            t = const.tile([P, 8 * nblk], I16, tag=f"sidx{s}")
            nc.sync.dma_start(out=t[:], in_=sidx_t[s].ap())
            sidx_sb.append(t)

        # Prime engines on the upfront loads so steady-state instructions
        # carry at most one sync wait each.
        prime = const.tile([P, 1], F32, tag="pr1")
        nc.vector.tensor_copy(out=prime[:], in_=wgt_sb[0][:, :1])
        prime2 = const.tile([P, 1], F32, tag="pr2")
        nc.scalar.activation(prime2[:], wgt_sb[0][:, :1],
                             mybir.ActivationFunctionType.Copy)
        prime_ps = psum.tile([P, P], F32, tag="tp")
        nc.tensor.transpose(out=prime_ps[:], in_=ident[:], identity=ident[:])
        nc.tensor.transpose(out=prime_ps[:D, :D], in_=wt_sb[:], identity=ident[:D, :D])

        regs = {}
        def nreg(v):
            if v not in regs:
                regs[v] = nc.gpsimd.to_reg(v)
            return regs[v]

        nsc = nblk * P  # scatter rows per shard (incl trailing pads)
        for s in range(NSH):
            delta = dpool.tile([P, nblk * D], F32, tag="d")
            for (b0, nb, off, width) in groups[s]:
                ni = width * P
                g = gpool.tile([P, width * D], F32, tag="g")
                nc.gpsimd.dma_gather(
                    g[:].rearrange("p (c d) -> p c d", d=D),
                    xc_t[s].ap(),
                    idx_sb[s][:, 8 * off : 8 * (off + width)],
                    ni, nreg(ni), D, elem_step=D,
                    single_packet=False,
                )
                msg = mpool.tile([P, width * D], F32, tag="m")
                nc.vector.tensor_tensor(
                    out=msg[:].rearrange("p (c d) -> p c d", d=D),
                    in0=g[:].rearrange("p (c d) -> p c d", d=D),
                    in1=wgt_sb[s][:, off : off + width].to_broadcast([P, width, D]),
                    op=mybir.AluOpType.mult,
                )
                bo = 0
                for j in range(b0, b0 + nb):
                    dj = int(d_sched[s][j])
                    agg = apool.tile([P, D], F32, tag="a")
                    nc.vector.tensor_reduce(
                        out=agg[:],
                        in_=msg[:, bo * D : (bo + dj) * D].rearrange(
                            "p (d o) -> p o d", o=D),
                        axis=mybir.AxisListType.X,
                        op=mybir.AluOpType.add,
                    )
                    agg_ps = psum.tile([D, P], F32, tag="tp")
                    nc.tensor.transpose(out=agg_ps[:], in_=agg[:], identity=ident[:])
                    agg_tr = tpool.tile([D, P], F32, tag="aT")
                    nc.scalar.activation(agg_tr[:], agg_ps[:],
                                         mybir.ActivationFunctionType.Copy)
                    d_ps = psum.tile([P, D], F32, tag="mm")
                    nc.tensor.matmul(out=d_ps[:], lhsT=agg_tr[:], rhs=wt_sb[:],
                                     start=True, stop=True)
                    nc.scalar.activation(delta[:, j * D : (j + 1) * D], d_ps[:],
                                         mybir.ActivationFunctionType.Copy)
                    bo += dj
            nc.gpsimd.dma_scatter_add(
                out_t.ap(),
                delta[:].rearrange("p (c d) -> p c d", d=D),
                sidx_sb[s][:],
                nsc, nreg(tps), D, elem_step=D,
                single_packet=False,
            )
    nc.compile()
    return nc


def run_gcn(x, W, edge_weights, source, target, num_nodes, trace=False, bufs=3):
    """Full-input host entry: preprocess, build, run on 8 cores, assemble."""
    n_nodes = int(num_nodes)
    pp = preprocess(x, source, target, edge_weights, n_nodes)
    nc = build_nc(pp["d_sched"], pp["groups"], pp["S"], pp["ncomp"],
                  pp["nt"], pp["tps"], pp["nblk"], bufs=bufs)
    wt_np = np.ascontiguousarray(np.asarray(W).T, dtype=np.float32)
    eye = np.eye(P, dtype=np.float32)
    in_maps = []
    for k in range(N_CORES):
        im = dict(pp["in_maps"][k])
        im["wT"] = wt_np
        im["eye"] = eye
        in_maps.append(im)
    res = run_bass_kernel_spmd(nc, in_maps, core_ids=list(range(N_CORES)),
                               trace=trace)
    out = np.concatenate([res.results[k]["out"] for k in range(N_CORES)], axis=0)
    return out, res


def kernel(**inputs) -> np.ndarray:
    """Harness entry: full unsharded inputs -> full (num_nodes, 64) output."""
    out, _ = run_gcn(
        np.asarray(inputs["x"]),
        np.asarray(inputs["W"]),
        np.asarray(inputs["edge_weights"]),
        np.asarray(inputs["source"]),
        np.asarray(inputs["target"]),
        int(inputs["num_nodes"]),
        trace=False,
    )
    return out
